# revision 1
# baseline (speedup 1.0000x reference)
"""GroupQueryAttention (B=1, S=2048, H=4096, 32 Q heads, 8 KV groups) on 8
Trainium2 NeuronCores, tensor-parallel over heads.

Sharding: core c owns Q heads 4c..4c+3 and KV group c. The reference's head
merge is `swapaxes(-1,-2).reshape`, which makes output row r = 64*h + d//2 and
column 2048*(d%2) + s -- i.e. each output row depends on exactly one head, so
the o-projection is row-parallel across cores with NO cross-core reduction.
Core c produces output rows [256c, 256c+256).

Device math per core (all matmuls bf16 with fp32 PSUM accumulation):
  Q^T[d,s]   = wq_c^T @ X^T        (1/sqrt(d) and bq folded into wq on host)
  K^T[d,s]   = wk_c^T @ X^T
  V[s,d]     = X @ wv_c            (lhsT = X^T k-tiles)
  S^T[sk,sq] = (K^T sk-slice)^T-contraction: lhsT=K^T[:,sk], rhs=Q^T[:,sq]
  E = exp(S^T)                     (no max subtraction: |scores| <~ 10)
  N[sq,0:128] = sum_sk E^T V ; N[sq,128] = denom (ones column appended to V)
  O = N[:, :128] * (1/N[:,128])    stored interleaved by d-parity for o-proj
  out rows = Y_c @ wo + bo         (Y_c^T k-tiles are strided views of O)
Biases enter via one extra contraction tile (row 4096 of padded operands = bias,
paired with an all-ones row-0 operand on the other side).
"""

import math
from contextlib import ExitStack

import ml_dtypes
import numpy as np

P = 128
S = 2048
HID = 4096
KPAD = HID + P          # 4224: one extra k-tile for the bias trick
KT = KPAD // P          # 33 contraction tiles
CH = 4                  # 512-wide seq chunks
SKT = 16                # 128-row sk tiles
NCORES = 8
HPC = 4                 # heads per core
BF16 = ml_dtypes.bfloat16

_CACHE = {}

PROFILE = False         # set True (e.g. from test.py) to capture an NTFF trace


def _build_nc():
    import concourse.tile as tile
    from concourse import bacc, mybir

    f32 = mybir.dt.float32
    bf16 = mybir.dt.bfloat16
    Exp = mybir.ActivationFunctionType.Exp
    Mult = mybir.AluOpType.mult

    nc = bacc.Bacc("TRN2", target_bir_lowering=False, debug=False)

    xt_d = nc.dram_tensor("xt", [KPAD, S], bf16, kind="ExternalInput").ap()
    wq_d = nc.dram_tensor("wq", [KPAD, 512], bf16, kind="ExternalInput").ap()
    wk_d = nc.dram_tensor("wk", [KPAD, P], bf16, kind="ExternalInput").ap()
    wv_d = nc.dram_tensor("wv", [KPAD, P], bf16, kind="ExternalInput").ap()
    wo_d = nc.dram_tensor("wo", [KPAD, HID], bf16, kind="ExternalInput").ap()
    ones_d = nc.dram_tensor("ones", [P, 512], bf16, kind="ExternalInput").ap()
    out_d = nc.dram_tensor("out", [2 * P, HID], f32, kind="ExternalOutput").ap()

    with tile.TileContext(nc) as tc, ExitStack() as ctx:
        pers = ctx.enter_context(tc.tile_pool(name="pers", bufs=1))
        attn = ctx.enter_context(tc.tile_pool(name="attn", bufs=1))
        psum = ctx.enter_context(tc.tile_pool(name="psum", bufs=1, space="PSUM"))

        # Persistent SBUF tensors
        qt = [pers.tile([P, S], bf16, name=f"qt{h}", tag=f"qt{h}") for h in range(HPC)]
        ktr = pers.tile([P, S], bf16, name="ktr", tag="ktr")
        # V with a ones column at 128 so the PV matmul also emits the denom
        v1 = pers.tile([P, SKT, 132], bf16, name="v1", tag="v1")
        # O interleaved: [s_local, pair, s_tile, d%2, head_in_pair, d//2]
        opair = pers.tile([P, 2, SKT, 2, 2, 64], bf16, name="opair", tag="opair")
        ones_sb = pers.tile([P, 512], bf16, name="ones_sb", tag="ones_sb")
        wv = pers.tile([P, KT, P], bf16, name="wv", tag="wv")

        nc.sync.dma_start(ones_sb[:], ones_d[:])
        nc.vector.memset(v1[:, :, 128:129], 1.0)

        with tc.tile_pool(name="qkv", bufs=1) as qkv:
            # xt tiles allocated up front; DMAs issued inline with first use
            # (chunk 0 of Q) so PE starts within a few us of kernel start.
            xt = [
                qkv.tile([P, S], bf16, name=f"xt{k}", tag=f"xt{k}")
                for k in range(KT - 1)
            ]

            def xrhs(k, c):
                # rhs [128,512] for contraction tile k, seq chunk c; tile 32 is
                # the bias tile: row 4096 of X^T_pad is all-ones.
                if k < KT - 1:
                    return xt[k][:, c * 512:(c + 1) * 512]
                return ones_sb[:]

            # ---- Q^T (one [128,2048] tile per head) ----
            for c in range(CH):
                qp = [
                    psum.tile([P, 512], f32, name=f"qp{h}", tag="acc", bufs=5)
                    for h in range(HPC)
                ]
                for k in range(KT):
                    if c == 0 and k < KT - 1:
                        nc.sync.dma_start(xt[k][:], xt_d[k * P:(k + 1) * P, :])
                    wqt = qkv.tile([P, 512], bf16, name="wqt", tag="wqt", bufs=4)
                    nc.sync.dma_start(wqt[:], wq_d[k * P:(k + 1) * P, :])
                    for h in range(HPC):
                        nc.tensor.matmul(
                            qp[h][:], wqt[:, h * P:(h + 1) * P], xrhs(k, c),
                            start=(k == 0), stop=(k == KT - 1),
                        )
                for h in range(HPC):
                    nc.vector.tensor_copy(qt[h][:, c * 512:(c + 1) * 512], qp[h][:])

            # ---- K^T ----
            for c in range(CH):
                kp = psum.tile([P, 512], f32, name="kp", tag="acc", bufs=5)
                for k in range(KT):
                    wkt = qkv.tile([P, P], bf16, name="wkt", tag="wkt", bufs=4)
                    nc.sync.dma_start(wkt[:], wk_d[k * P:(k + 1) * P, :])
                    nc.tensor.matmul(
                        kp[:], wkt[:], xrhs(k, c),
                        start=(k == 0), stop=(k == KT - 1),
                    )
                nc.vector.tensor_copy(ktr[:, c * 512:(c + 1) * 512], kp[:])

            # ---- V ----
            for k in range(KT):
                nc.sync.dma_start(wv[:, k, :], wv_d[k * P:(k + 1) * P, :])
            for sk in range(SKT):
                vp = psum.tile([P, 512], f32, name="vp", tag="acc", bufs=5)
                for k in range(KT):
                    if k < KT - 1:
                        lhs = xt[k][:, sk * P:(sk + 1) * P]
                    else:
                        lhs = ones_sb[:, :P]
                    nc.tensor.matmul(
                        vp[:, :P], lhs, wv[:, k, :],
                        start=(k == 0), stop=(k == KT - 1),
                    )
                nc.vector.tensor_copy(v1[:, sk, :P], vp[:, :P])

        # ---- attention: scores^T -> exp -> (E^T)V with fused denominator ----
        for h in range(HPC):
            pair, j = divmod(h, 2)
            for c in range(CH):
                nps = [
                    psum.tile([P, 512], f32, name=f"np{q}", tag="acc", bufs=5)
                    for q in range(4)
                ]
                for sk in range(SKT):
                    sp = psum.tile([P, 512], f32, name="sp", tag="sp", bufs=3)
                    nc.tensor.matmul(
                        sp[:], ktr[:, sk * P:(sk + 1) * P],
                        qt[h][:, c * 512:(c + 1) * 512],
                        start=True, stop=True,
                    )
                    es = attn.tile([P, 512], bf16, name="es", tag="es", bufs=8)
                    nc.scalar.activation(es[:], sp[:], Exp)
                    for q in range(4):
                        nc.tensor.matmul(
                            nps[q][:, :129], es[:, q * P:(q + 1) * P],
                            v1[:, sk, :129],
                            start=(sk == 0), stop=(sk == SKT - 1),
                        )
                for q in range(4):
                    st = c * 4 + q
                    rc = attn.tile([P, 1], f32, name="rc", tag="rc", bufs=4)
                    nc.vector.reciprocal(rc[:], nps[q][:, 128:129])
                    for par in range(2):
                        nc.vector.tensor_scalar(
                            opair[:, pair, st, par, j, :],
                            nps[q][:, par:P:2], rc[:], None, Mult,
                        )

        # ---- o-projection: out rows (h,d//2) = Y_c @ wo_pad ----
        with tc.tile_pool(name="oproj", bufs=1) as op:
            out_sb = [
                op.tile([P, HID], f32, name=f"osb{mt}", tag=f"osb{mt}")
                for mt in range(2)
            ]
            for blk in range(4):
                ops = [
                    psum.tile([P, 512], f32, name=f"op{i}", tag="acc", bufs=5)
                    for i in range(4)
                ]
                for k in range(KT):
                    wot = op.tile([P, 1024], bf16, name="wot", tag="wot", bufs=12)
                    nc.sync.dma_start(
                        wot[:], wo_d[k * P:(k + 1) * P, blk * 1024:(blk + 1) * 1024]
                    )
                    st, par = k % SKT, k // SKT
                    for mt in range(2):
                        if k < KT - 1:
                            lhs = opair[:, mt, st, par, :, :]
                        else:
                            lhs = ones_sb[:, :P]
                        for cc in range(2):
                            nc.tensor.matmul(
                                ops[mt * 2 + cc][:], lhs,
                                wot[:, cc * 512:(cc + 1) * 512],
                                start=(k == 0), stop=(k == KT - 1),
                            )
                for mt in range(2):
                    for cc in range(2):
                        nc.vector.tensor_copy(
                            out_sb[mt][:, blk * 1024 + cc * 512:
                                       blk * 1024 + (cc + 1) * 512],
                            ops[mt * 2 + cc][:],
                        )
                    nc.sync.dma_start(
                        out_d[mt * P:(mt + 1) * P, blk * 1024:(blk + 1) * 1024],
                        out_sb[mt][:, blk * 1024:(blk + 1) * 1024],
                    )

    nc.compile()
    return nc


def _get_nc():
    if "nc" not in _CACHE:
        _CACHE["nc"] = _build_nc()
    return _CACHE["nc"]


def kernel(hidden_state, wq, bq, wk, bk, wv, bv, wo, bo):
    from concourse import bass_utils

    nc = _get_nc()

    X = np.asarray(hidden_state, np.float32).reshape(S, HID)
    scale = 1.0 / math.sqrt(P)

    xt_pad = np.zeros((KPAD, S), np.float32)
    xt_pad[:HID] = X.T
    xt_pad[HID] = 1.0
    xt_bf = xt_pad.astype(BF16)

    wo_pad = np.zeros((KPAD, HID), np.float32)
    wo_pad[:HID] = np.asarray(wo, np.float32)
    wo_pad[HID] = np.asarray(bo, np.float32)
    wo_bf = wo_pad.astype(BF16)

    ones_np = np.zeros((P, 512), np.float32)
    ones_np[0] = 1.0
    ones_bf = ones_np.astype(BF16)

    wq = np.asarray(wq, np.float32)
    bq = np.asarray(bq, np.float32)
    wk = np.asarray(wk, np.float32)
    bk = np.asarray(bk, np.float32)
    wv = np.asarray(wv, np.float32)
    bv = np.asarray(bv, np.float32)

    in_maps = []
    for c in range(NCORES):
        wq_pad = np.zeros((KPAD, 512), np.float32)
        wq_pad[:HID] = wq[:, c * 512:(c + 1) * 512] * scale
        wq_pad[HID] = bq[c * 512:(c + 1) * 512] * scale
        wk_pad = np.zeros((KPAD, P), np.float32)
        wk_pad[:HID] = wk[:, c * P:(c + 1) * P]
        wk_pad[HID] = bk[c * P:(c + 1) * P]
        wv_pad = np.zeros((KPAD, P), np.float32)
        wv_pad[:HID] = wv[:, c * P:(c + 1) * P]
        wv_pad[HID] = bv[c * P:(c + 1) * P]
        in_maps.append({
            "xt": xt_bf,
            "wq": wq_pad.astype(BF16),
            "wk": wk_pad.astype(BF16),
            "wv": wv_pad.astype(BF16),
            "wo": wo_bf,
            "ones": ones_bf,
        })

    try:
        res = bass_utils.run_bass_kernel_spmd(
            nc, in_maps, core_ids=list(range(NCORES)), trace=PROFILE,
        )
    except ModuleNotFoundError:
        # NTFF profile hook unavailable in this environment
        res = bass_utils.run_bass_kernel_spmd(
            nc, in_maps, core_ids=list(range(NCORES)), trace=False,
        )
    _CACHE["last_results"] = res

    out = np.empty((1, S, HID), np.float32)
    for c in range(NCORES):
        out[0, c * 256:(c + 1) * 256, :] = res.results[c]["out"]
    return out

